# revision 1
# baseline (speedup 1.0000x reference)
"""Bass/Tile TRN2 kernel for nn_CRMF_35296041239144.

Social-LSTM-style decoder: mapping MLP on K x B hidden states, then a
12-step LSTM recurrence (hard-sigmoid gates, clipped cell) with a 2-dim
output projection per step.

Sharding: batch 2048 -> 8 cores x 256. Per core rows = K*Bc = 5120.
State is kept transposed [H=128 partitions, rows free]; the host
pre-transposes pred_lstm_hidden so no on-device transposes are needed.
h/c state and all elementwise tiles are bf16 (2x/4x DVE modes); matmul
stationary weights stay f32r, moving operands are bf16 (full PE rate).

hard_sigmoid(z) = clip(z/6 + 0.5, 0, 1): the 1/6 scale and +0.5 offset
are pre-folded into the i/f/o weight blocks host-side; the x-term + bias
ride in contraction-3 matmuls placed in distinct PE row-groups so all
four run concurrently.

Final structure (after 11 same-session-benchmarked passes): per step,
per-chunk matmul bursts (gates + x-term, out-proj MMs col-tiled 4-up
into one persistent PSUM bank at partitions {32l,32l+1}, 3 ACT evacs
per step); ACT evacs i,f,o as one Relu instr per chunk; the post-evac
chain runs entirely on DVE per chunk-PAIR at [128,1024] (gt clamps
straight from PSUM, prefetched GT_DIST=2 pairs ahead to free the
single psg bank early; t1/t2/c-add/c-clamp/h as bf16 ops on pair
tiles). Same-engine grouping + early PSUM freeing are the two rules
that every winning emission order obeys; see the project memory ledger
for the full experiment history (8 shipped wins, 12 falsified).
"""

import numpy as np
from contextlib import ExitStack, nullcontext

import concourse.bass as bass
import concourse.bacc as bacc
import concourse.tile as tile
from concourse import mybir
from concourse.bass_utils import run_bass_kernel_spmd
import concourse.bass_utils as _bass_utils

# NOTE: the baseline's --enable-ldw-opt=true patch is dropped: it is
# incompatible with bf16 LDWEIGHTS (walrus rejects), and bf16 weights get
# FWL (fast weight load) automatically.

OBS_LEN, K, B, H, MID, NC_OUT, CIN = 12, 20, 2048, 128, 256, 2, 3
NCORES = 8
BC = B // NCORES            # 256 batch rows per core
ROWS = K * BC               # 5120 rows per core (k-major: r = k*BC + b)
CHUNK = 512
NCH = ROWS // CHUNK         # 10

F32 = mybir.dt.float32
F32R = mybir.dt.float32r
BF16 = mybir.dt.bfloat16
AF = mybir.ActivationFunctionType
OP = mybir.AluOpType

# gate order used on device: [i, f, o, g]; source block order in w_ih/w_hh
# is [i, f, g, o] (reference splits gates into i,f,g,o).
SRC_BLOCK = [0, 1, 3, 2]

# engine assignment switches for the post-evac chain
CC_ENGINE = 'dve'   # 'dve' | 'gpsimd'   c-clamp
T1_ENGINE = 'dve'      # 'dve' | 'gpsimd'   t1 = min(f,1)*c
CADD_ENGINE = 'dve'    # 'dve' | 'gpsimd'   c = t1 + t2
OUT_EVAC = 'act'       # 'act' | 'dve'      out-projection evac
GT_MODE = 'dve_psum'       # 'dve_psum' | 'act_copy' | 'alt' | 'alt3'
O_SPLIT = False        # odd chunks: ACT evacs i,f only; DVE clips o
PAIR_DVE = True        # run post-evac DVE chain on chunk PAIRS [128,1024]
PAIR_MM = False        # gate-major MM emission within pairs (LDW reuse)
GT_EARLY = True        # emit pair p+1's gt clamps before pair p's group
GT_DIST = 2            # how many pairs ahead to prefetch gt clamps
OE_LATE = False        # emit out-evacs in the evac phase (no ACT HOL stall)
OE_INLINE = False      # evacs emitted inline in the MM loop: ACT order
                       # [ifo(0..3), oe(g0), ifo(4..7), oe(g1), ...] with
                       # correct psout WAR binding (oe(g) before oMM(g+1))
OUT_BF16 = False       # bf16 out-projection dest (halves ACT + DMA cost)
OUT_GROUPED = True     # col-tiled 4-up out-proj MMs into one PSUM bank
OUT_GROUPS = [(0, [0, 1, 2, 3]), (1, [4, 5, 6, 7]), (2, [8, 9])]
OUT_DMA = 'fine'       # 'coarse': [128,512] per group, host extracts
                       # 'fine': [2,512] per chunk


def build_nc(reps: int = 1):
    nc = bacc.Bacc("TRN2", target_bir_lowering=False, debug=False)

    pht = nc.dram_tensor("pht", [H, ROWS], BF16, kind="ExternalInput")
    xr = nc.dram_tensor("xr", [3, OBS_LEN, CHUNK], BF16, kind="ExternalInput")
    whh = nc.dram_tensor("whh", [H, 4 * H], BF16, kind="ExternalInput")
    wih = nc.dram_tensor("wih", [128, H], BF16, kind="ExternalInput")
    w0 = nc.dram_tensor("w0", [H, MID], BF16, kind="ExternalInput")
    w1 = nc.dram_tensor("w1", [MID, H], BF16, kind="ExternalInput")
    oww = nc.dram_tensor("oww", [H, NC_OUT], BF16, kind="ExternalInput")
    bpack = nc.dram_tensor("bpack", [128, 4], F32, kind="ExternalInput")
    if OUT_GROUPED and OUT_DMA == 'coarse':
        outd = nc.dram_tensor("out", [OBS_LEN, len(OUT_GROUPS), 128, CHUNK],
                              F32, kind="ExternalOutput")
    else:
        outd = nc.dram_tensor("out", [OBS_LEN, NC_OUT, ROWS],
                              BF16 if OUT_BF16 else F32,
                              kind="ExternalOutput")

    with tile.TileContext(nc) as tc:
        with tc.tile_pool(name="const", bufs=1) as const, \
             tc.tile_pool(name="state", bufs=1) as state, \
             tc.tile_pool(name="outs", bufs=2) as outs_p:

            whh_sb = const.tile([128, 4 * H], BF16)
            nc.sync.dma_start(out=whh_sb[:], in_=whh[:])
            wih_sb = const.tile([128, H], BF16)
            nc.sync.dma_start(out=wih_sb[:], in_=wih[:])
            w0_sb = const.tile([128, MID], BF16)
            nc.sync.dma_start(out=w0_sb[:], in_=w0[:])
            w1_sb = const.tile([128, 2, H], BF16)
            nc.sync.dma_start(out=w1_sb[:],
                              in_=w1.rearrange("(a p) h -> p a h", p=128))
            oww_sb = const.tile([128, NC_OUT], BF16)
            nc.sync.dma_start(out=oww_sb[:], in_=oww[:])
            bp_sb = const.tile([128, 4], F32)
            nc.sync.dma_start(out=bp_sb[:], in_=bpack[:])
            # x-term moving operand, replicated into 4 partition groups
            xr_sb = const.tile([128, OBS_LEN, CHUNK], BF16)
            for g in range(4):
                nc.sync.dma_start(out=xr_sb[32 * g:32 * g + 3, :, :],
                                  in_=xr[:])

            psout = None
            _psout_stack = ExitStack()
            if OUT_GROUPED:
                psout_pool = _psout_stack.enter_context(
                    tc.tile_pool(name="psout", bufs=1, space="PSUM"))
                psout = psout_pool.tile([128, CHUNK], F32, name="psout")
                nc.vector.memset(psout[:], 0.0)

            # per-chunk state tiles so chunk pipelines stay independent
            if PAIR_DVE:
                h_pr = [state.tile([128, 2, CHUNK], BF16, name=f"hp{p}",
                                   tag=f"hp{p}") for p in range(NCH // 2)]
                c_pr = [state.tile([128, 2, CHUNK], BF16, name=f"cp{p}",
                                   tag=f"cp{p}") for p in range(NCH // 2)]
                hAP = lambda j: h_pr[j // 2][:, j % 2, :]
                cAP = lambda j: c_pr[j // 2][:, j % 2, :]
            else:
                h_ch = [state.tile([128, CHUNK], BF16, name=f"h{j}",
                                   tag=f"h{j}") for j in range(NCH)]
                c_ch = [state.tile([128, CHUNK], BF16, name=f"c{j}",
                                   tag=f"c{j}") for j in range(NCH)]
                hAP = lambda j: h_ch[j][:]
                cAP = lambda j: c_ch[j][:]
            pht_sb = state.tile([128, ROWS], BF16, tag="pht")

            with (tc.For_i(0, reps, 1) if reps > 1 else nullcontext()):
                # ---------- phase 1: mapping MLP (ph pre-transposed) ------
                with tc.tile_pool(name="h1p", bufs=3) as h1p, \
                     tc.tile_pool(name="ps1", bufs=2, space="PSUM") as ps1p, \
                     tc.tile_pool(name="ps0", bufs=2, space="PSUM") as ps0p:

                    if PAIR_DVE:
                        for p in range(NCH // 2):
                            nc.gpsimd.memset(c_pr[p][:], 0.0)
                    else:
                        for j in range(NCH):
                            nc.gpsimd.memset(c_ch[j][:], 0.0)
                    for j in range(NCH):
                        nc.sync.dma_start(
                            out=pht_sb[:, j * CHUNK:(j + 1) * CHUNK],
                            in_=pht[:, j * CHUNK:(j + 1) * CHUNK])

                    ps1s, h1ts, ps0s = [], [], []
                    for j in range(NCH):
                        rs = slice(j * CHUNK, (j + 1) * CHUNK)
                        ps1 = ps1p.tile([128, 2, CHUNK], F32)
                        ps1s.append(ps1)
                        nc.tensor.matmul(ps1[:, 0, :], w0_sb[:, 0:128],
                                         pht_sb[:, rs], start=True,
                                         stop=True)
                        nc.tensor.matmul(ps1[:, 1, :], w0_sb[:, 128:256],
                                         pht_sb[:, rs], start=True,
                                         stop=True)
                    for j in range(NCH):
                        ps1 = ps1s[j]
                        h1t = h1p.tile([128, 2, CHUNK], BF16, tag="h1")
                        h1ts.append(h1t)
                        nc.scalar.activation(h1t[:, 0, :], ps1[:, 0, :],
                                             AF.Lrelu, bias=bp_sb[:, 0:1],
                                             alpha=0.01)
                        nc.scalar.activation(h1t[:, 1, :], ps1[:, 1, :],
                                             AF.Lrelu, bias=bp_sb[:, 1:2],
                                             alpha=0.01)
                        ps0 = ps0p.tile([128, CHUNK], F32)
                        ps0s.append(ps0)
                        nc.tensor.matmul(ps0[:], w1_sb[:, 0, :],
                                         h1t[:, 0, :], start=True,
                                         stop=False)
                        nc.tensor.matmul(ps0[:], w1_sb[:, 1, :],
                                         h1t[:, 1, :], start=False,
                                         stop=True)
                    for j in range(NCH):
                        nc.vector.tensor_scalar(
                            out=hAP(j), in0=ps0s[j][:],
                            scalar1=bp_sb[:, 2:3], scalar2=None, op0=OP.add)

                # ---------- phase 2: LSTM recurrence ----------
                with tc.tile_pool(name="psifo", bufs=2,
                                  space="PSUM") as psifo_p, \
                     tc.tile_pool(name="psg", bufs=1, space="PSUM") as psg_p, \
                     tc.tile_pool(name="pso", bufs=1, space="PSUM") as pso_p, \
                     tc.tile_pool(name="gsb", bufs=12) as gsb_p, \
                     tc.tile_pool(name="tmp", bufs=12) as tmp_p:

                    def out_proj(j, dest):
                        pso = pso_p.tile([NC_OUT, CHUNK], F32, name="pso")
                        nc.tensor.matmul(pso[:], oww_sb[:], hAP(j),
                                         start=True, stop=True)
                        if OUT_EVAC == 'act':
                            nc.scalar.activation(
                                dest[:, j * CHUNK:(j + 1) * CHUNK], pso[:],
                                AF.Identity, bias=bp_sb[0:NC_OUT, 3:4])
                        else:
                            nc.vector.tensor_scalar(
                                out=dest[:, j * CHUNK:(j + 1) * CHUNK],
                                in0=pso[:], scalar1=bp_sb[0:NC_OUT, 3:4],
                                scalar2=None, op0=OP.add)

                    def out_proj_mms(g, lanes):
                        # 4 chunks' [2,512] out-MMs col-tiled into one PSUM
                        # bank at partitions {32l, 32l+1}
                        for l, j in enumerate(lanes):
                            nc.tensor.matmul(
                                psout[32 * l:32 * l + NC_OUT, :],
                                oww_sb[:], hAP(j),
                                start=True, stop=True,
                                tile_position=(0, 32 * l))

                    def out_proj_evac(g, dest):
                        nc.scalar.activation(dest[:, g, :], psout[:],
                                             AF.Identity,
                                             bias=bp_sb[:, 3:4])

                    def out_proj_group(g, lanes, dest):
                        out_proj_mms(g, lanes)
                        out_proj_evac(g, dest)

                    outstep = None
                    prev_outstep = None
                    for t in range(OBS_LEN):
                        prev_outstep = outstep
                        if OUT_GROUPED:
                            outstep = outs_p.tile([128, len(OUT_GROUPS),
                                                   CHUNK], F32,
                                                  tag="outstep",
                                                  name="outstep")
                        else:
                            outstep = outs_p.tile([NC_OUT, ROWS],
                                                  BF16 if OUT_BF16 else F32,
                                                  tag="outstep",
                                                  name="outstep")
                        # -- A+B: per chunk: out-proj of the PREVIOUS
                        # step's h (reads h before this step's elementwise
                        # overwrites it), then all gate matmuls + x-term.
                        # The out ping-pong (single pso bank) hides under
                        # the gate MMs.
                        psifos, psgs, gts, ifos = [], [], [], []
                        graws, pair_ifos = [], []
                        xop = xr_sb[:, t, :]

                        def emit_evacs(j):
                            gt_act = (GT_MODE == 'act_copy'
                                      or (GT_MODE == 'alt' and j % 2 == 0)
                                      or (GT_MODE == 'alt3' and j % 3 != 2))
                            if gt_act:
                                graw = gsb_p.tile([128, CHUNK], BF16,
                                                  tag="graw", name="graw")
                                nc.scalar.activation(graw[:], psgs[j][:],
                                                     AF.Copy)
                                graws.append(graw)
                            else:
                                graws.append(None)
                            if j % 2 == 0:
                                pair_ifos.append(gsb_p.tile(
                                    [128, 2, 3, CHUNK], BF16,
                                    tag="ifo", name="ifo", bufs=6))
                            nc.scalar.activation(
                                pair_ifos[j // 2][:, j % 2, :, :],
                                psifos[j][:], AF.Relu)
                            if t > 0 and j in (3, 7, 9):
                                out_proj_evac(min(j // 4, 2), prev_outstep)

                        for j in range(NCH):
                            if t > 0:
                                if OUT_GROUPED:
                                    if j % 4 == 0 and j // 4 < len(OUT_GROUPS):
                                        g, lanes = OUT_GROUPS[j // 4]
                                        if OE_LATE:
                                            out_proj_mms(g, lanes)
                                        else:
                                            out_proj_group(g, lanes,
                                                           prev_outstep)
                                else:
                                    out_proj(j, prev_outstep)
                            psifo = psifo_p.tile([128, 3, CHUNK], F32,
                                                 name="psifo")
                            psifos.append(psifo)
                            psg = psg_p.tile([128, CHUNK], F32, name="psg")
                            psgs.append(psg)
                            if PAIR_MM:
                                if j % 2 == 0:
                                    continue    # emit at odd j, pair-major
                                for gi in range(3):
                                    for jj in (j - 1, j):
                                        nc.tensor.matmul(
                                            psifos[jj][:, gi, :],
                                            whh_sb[:, gi * 128:(gi + 1) * 128],
                                            hAP(jj), start=True, stop=False)
                                for jj in (j - 1, j):
                                    nc.tensor.matmul(
                                        psgs[jj][:], whh_sb[:, 384:512],
                                        hAP(jj), start=True, stop=False)
                                for jj in (j - 1, j):
                                    for gi in range(3):
                                        nc.tensor.matmul(
                                            psifos[jj][:, gi, :],
                                            wih_sb[32 * gi:32 * gi + 3, :],
                                            xop[32 * gi:32 * gi + 3, :],
                                            start=False, stop=True,
                                            tile_position=(32 * gi, 0))
                                    nc.tensor.matmul(
                                        psgs[jj][:], wih_sb[96:99, :],
                                        xop[96:99, :],
                                        start=False, stop=True,
                                        tile_position=(96, 0))
                                continue
                            for gi in range(3):
                                nc.tensor.matmul(
                                    psifo[:, gi, :],
                                    whh_sb[:, gi * 128:(gi + 1) * 128],
                                    hAP(j), start=True, stop=False)
                            nc.tensor.matmul(psg[:], whh_sb[:, 384:512],
                                             hAP(j),
                                             start=True, stop=False)
                            for gi in range(3):
                                nc.tensor.matmul(
                                    psifo[:, gi, :],
                                    wih_sb[32 * gi:32 * gi + 3, :],
                                    xop[32 * gi:32 * gi + 3, :],
                                    start=False, stop=True,
                                    tile_position=(32 * gi, 0))
                            nc.tensor.matmul(
                                psg[:], wih_sb[96:99, :], xop[96:99, :],
                                start=False, stop=True,
                                tile_position=(96, 0))
                            if OE_INLINE and PAIR_DVE:
                                emit_evacs(j)
                        # -- C: PSUM evacs (ACT): g copy + i,f,o Relu
                        for j in range(NCH):
                            if OE_INLINE and PAIR_DVE:
                                break
                            gt_act = (GT_MODE == 'act_copy'
                                      or (GT_MODE == 'alt' and j % 2 == 0)
                                      or (GT_MODE == 'alt3' and j % 3 != 2))
                            if gt_act:
                                graw = gsb_p.tile([128, CHUNK], BF16,
                                                  tag="graw", name="graw")
                                nc.scalar.activation(graw[:], psgs[j][:],
                                                     AF.Copy)
                                graws.append(graw)
                            else:
                                graws.append(None)
                            if PAIR_DVE:
                                if j % 2 == 0:
                                    pair_ifos.append(gsb_p.tile(
                                        [128, 2, 3, CHUNK], BF16,
                                        tag="ifo", name="ifo", bufs=6))
                                nc.scalar.activation(
                                    pair_ifos[j // 2][:, j % 2, :, :],
                                    psifos[j][:], AF.Relu)
                                if OE_LATE and t > 0 and j in (3, 7, 9):
                                    out_proj_evac(min(j // 4, 2),
                                                  prev_outstep)
                                continue
                            ifo = gsb_p.tile([128, 3, CHUNK], BF16,
                                             tag="ifo", name="ifo")
                            ifos.append(ifo)
                            if O_SPLIT and j % 2 == 1:
                                nc.scalar.activation(ifo[:, 0:2, :],
                                                     psifos[j][:, 0:2, :],
                                                     AF.Relu)
                            else:
                                nc.scalar.activation(ifo[:], psifos[j][:],
                                                     AF.Relu)
                        # -- D: per-chunk all-DVE gate chain (grouping per
                        # chunk keeps h(t,j) landing early so step t+1's
                        # matmuls overlap this step's tail)
                        if PAIR_DVE:
                            gtps = []

                            def emit_gts(p):
                                gtp = gsb_p.tile([128, 2, CHUNK], BF16,
                                                 tag="g", name="gtp",
                                                 bufs=6)
                                gtps.append(gtp)
                                for s, j in ((0, 2 * p), (1, 2 * p + 1)):
                                    srcap = (graws[j][:]
                                             if graws[j] is not None
                                             else psgs[j][:])
                                    nc.vector.tensor_scalar(
                                        out=gtp[:, s, :], in0=srcap,
                                        scalar1=1.0, scalar2=-1.0,
                                        op0=OP.min, op1=OP.max)

                            if GT_EARLY:
                                for d in range(min(GT_DIST, NCH // 2)):
                                    emit_gts(d)
                            for p in range(NCH // 2):
                                if GT_EARLY:
                                    if p + GT_DIST < NCH // 2:
                                        emit_gts(p + GT_DIST)
                                else:
                                    emit_gts(p)
                                gtp = gtps[p]
                                ifo = pair_ifos[p]
                                t1 = tmp_p.tile([128, 2, CHUNK], BF16,
                                                tag="t1", name="t1", bufs=6)
                                nc.vector.scalar_tensor_tensor(
                                    out=t1[:], in0=ifo[:, :, 1, :],
                                    scalar=1.0, in1=c_pr[p][:],
                                    op0=OP.min, op1=OP.mult)
                                t2 = tmp_p.tile([128, 2, CHUNK], BF16,
                                                tag="t2", name="t2", bufs=6)
                                nc.vector.scalar_tensor_tensor(
                                    out=t2[:], in0=ifo[:, :, 0, :],
                                    scalar=1.0, in1=gtp[:],
                                    op0=OP.min, op1=OP.mult)
                                nc.vector.tensor_tensor(
                                    out=c_pr[p][:], in0=t1[:], in1=t2[:],
                                    op=OP.add)
                                cc = tmp_p.tile([128, 2, CHUNK], BF16,
                                                tag="cc", name="cc", bufs=6)
                                nc.vector.tensor_scalar(
                                    out=cc[:], in0=c_pr[p][:], scalar1=1.0,
                                    scalar2=-1.0, op0=OP.min, op1=OP.max)
                                nc.vector.scalar_tensor_tensor(
                                    out=h_pr[p][:], in0=ifo[:, :, 2, :],
                                    scalar=1.0, in1=cc[:], op0=OP.min,
                                    op1=OP.mult)
                        else:
                          for j in range(NCH):
                            oc = None
                            if O_SPLIT and j % 2 == 1:
                                oc = tmp_p.tile([128, CHUNK], BF16,
                                                tag="oc", name="oc")
                                nc.vector.tensor_scalar(
                                    out=oc[:], in0=psifos[j][:, 2, :],
                                    scalar1=1.0, scalar2=0.0,
                                    op0=OP.min, op1=OP.max)
                            gt = gsb_p.tile([128, CHUNK], BF16, tag="g",
                                            name="gt")
                            if graws[j] is not None:
                                nc.vector.tensor_scalar(
                                    out=gt[:], in0=graws[j][:], scalar1=1.0,
                                    scalar2=-1.0, op0=OP.min, op1=OP.max)
                            else:
                                nc.vector.tensor_scalar(
                                    out=gt[:], in0=psgs[j][:], scalar1=1.0,
                                    scalar2=-1.0, op0=OP.min, op1=OP.max)
                            t1 = tmp_p.tile([128, CHUNK], BF16, tag="t1",
                                            name="t1")
                            nc.vector.scalar_tensor_tensor(
                                out=t1[:], in0=ifos[j][:, 1, :], scalar=1.0,
                                in1=c_ch[j][:], op0=OP.min, op1=OP.mult)
                            t2 = tmp_p.tile([128, CHUNK], BF16, tag="t2",
                                            name="t2")
                            nc.vector.scalar_tensor_tensor(
                                out=t2[:], in0=ifos[j][:, 0, :], scalar=1.0,
                                in1=gt[:], op0=OP.min, op1=OP.mult)
                            nc.vector.tensor_tensor(
                                out=c_ch[j][:], in0=t1[:], in1=t2[:],
                                op=OP.add)
                            cc = tmp_p.tile([128, CHUNK], BF16, tag="cc",
                                            name="cc")
                            nc.vector.tensor_scalar(
                                out=cc[:], in0=c_ch[j][:], scalar1=1.0,
                                scalar2=-1.0, op0=OP.min, op1=OP.max)
                            if oc is not None:
                                nc.vector.tensor_tensor(
                                    out=h_ch[j][:], in0=oc[:], in1=cc[:],
                                    op=OP.mult)
                            else:
                                nc.vector.scalar_tensor_tensor(
                                    out=h_ch[j][:], in0=ifos[j][:, 2, :],
                                    scalar=1.0, in1=cc[:], op0=OP.min,
                                    op1=OP.mult)
                        if t > 0:
                            if OUT_GROUPED:
                                if OUT_DMA == 'coarse':
                                    nc.sync.dma_start(
                                        out=outd[t - 1],
                                        in_=prev_outstep[:])
                                else:
                                    for g, lanes in OUT_GROUPS:
                                        for l, j in enumerate(lanes):
                                            nc.sync.dma_start(
                                                out=outd[t - 1][
                                                    :, j * CHUNK:(j + 1) * CHUNK],
                                                in_=prev_outstep[
                                                    32 * l:32 * l + NC_OUT, g, :])
                            else:
                                nc.sync.dma_start(out=outd[t - 1],
                                                  in_=prev_outstep[:])

                    if OUT_GROUPED:
                        for g, lanes in OUT_GROUPS:
                            out_proj_group(g, lanes, outstep)
                        if OUT_DMA == 'coarse':
                            nc.sync.dma_start(out=outd[OBS_LEN - 1],
                                              in_=outstep[:])
                        else:
                            for g, lanes in OUT_GROUPS:
                                for l, j in enumerate(lanes):
                                    nc.sync.dma_start(
                                        out=outd[OBS_LEN - 1][
                                            :, j * CHUNK:(j + 1) * CHUNK],
                                        in_=outstep[32 * l:32 * l + NC_OUT, g, :])
                    else:
                        for j in range(NCH):
                            out_proj(j, outstep)
                        nc.sync.dma_start(out=outd[OBS_LEN - 1],
                                          in_=outstep[:])

            _psout_stack.close()

    nc.finalize()
    return nc


def prep_inputs(obs_traj_rel, pred_lstm_hidden, map_w0, map_b0, map_w1,
                map_b1, w_ih, w_hh, b_ih, b_hh, out_w, out_b):
    """Host-side prep -> list of per-core input dicts."""
    f32 = np.float32
    bias = (np.asarray(b_ih, f32) + np.asarray(b_hh, f32))
    w_hh = np.asarray(w_hh, f32)
    w_ih = np.asarray(w_ih, f32)

    whh_stat = np.empty((H, 4 * H), f32)
    wih_stat = np.zeros((128, H), f32)
    for gi in range(4):
        sb = SRC_BLOCK[gi]
        s = (1.0 / 6.0) if gi < 3 else 1.0
        off = 0.5 if gi < 3 else 0.0
        whh_stat[:, gi * 128:(gi + 1) * 128] = \
            w_hh[sb * 128:(sb + 1) * 128].T * s
        wih_stat[32 * gi + 0:32 * gi + 2, :] = \
            w_ih[sb * 128:(sb + 1) * 128, :].T * s
        wih_stat[32 * gi + 2, :] = bias[sb * 128:(sb + 1) * 128] * s + off

    bpack = np.zeros((128, 4), f32)
    bpack[:, 0] = np.asarray(map_b0, f32)[0:128]
    bpack[:, 1] = np.asarray(map_b0, f32)[128:256]
    bpack[:, 2] = np.asarray(map_b1, f32)
    for l in range(4):
        bpack[32 * l:32 * l + NC_OUT, 3] = np.asarray(out_b, f32)

    obs = np.asarray(obs_traj_rel, f32)
    xs = np.concatenate([obs[0:1], obs[:-1]], axis=0)[:, :, 0:2]  # [T,B,2]
    ph_full = np.asarray(pred_lstm_hidden, f32)

    import ml_dtypes
    bf16 = ml_dtypes.bfloat16
    common = dict(
        whh=whh_stat.astype(bf16), wih=wih_stat.astype(bf16),
        w0=np.ascontiguousarray(np.asarray(map_w0, f32)).astype(bf16),
        w1=np.ascontiguousarray(np.asarray(map_w1, f32)).astype(bf16),
        oww=np.ascontiguousarray(np.asarray(out_w, f32)).astype(bf16),
        bpack=bpack,
    )
    in_maps = []
    for c in range(NCORES):
        bs = slice(c * BC, (c + 1) * BC)
        pht_core = np.ascontiguousarray(
            ph_full[:, bs, :].reshape(ROWS, H).T).astype(bf16)  # [H, ROWS]
        x_core = xs[:, bs, :]                       # [T, BC, 2]
        xr_core = np.empty((3, OBS_LEN, CHUNK), f32)
        for t in range(OBS_LEN):
            for rep in range(CHUNK // BC):
                xr_core[0, t, rep * BC:(rep + 1) * BC] = x_core[t, :, 0]
                xr_core[1, t, rep * BC:(rep + 1) * BC] = x_core[t, :, 1]
        xr_core[2] = 1.0
        in_maps.append(dict(pht=pht_core, xr=xr_core.astype(bf16), **common))
    return in_maps


def assemble_output(results):
    """Per-core device layout -> full [T, K, B, 2]."""
    out = np.empty((OBS_LEN, K, B, NC_OUT), np.float32)
    for c, res in enumerate(results):
        raw = np.asarray(res["out"], np.float32)
        if raw.ndim == 4:
            # raw [T, G, 128, 512]; chunk j=4g+l at partitions 32l..32l+1
            o = np.empty((OBS_LEN, NC_OUT, ROWS), np.float32)
            for g, lanes in OUT_GROUPS:
                for l, j in enumerate(lanes):
                    o[:, :, j * CHUNK:(j + 1) * CHUNK] = \
                        raw[:, g, 32 * l:32 * l + NC_OUT, :]
        else:
            o = raw
        o = o.reshape(OBS_LEN, NC_OUT, K, BC)
        out[:, :, c * BC:(c + 1) * BC, :] = o.transpose(0, 2, 3, 1)
    return out


def kernel(**inputs):
    nc = build_nc(reps=1)
    in_maps = prep_inputs(**inputs)
    res = run_bass_kernel_spmd(nc, in_maps, core_ids=list(range(NCORES)))
    return assemble_output(res.results)


if __name__ == "__main__":
    import reference as R
    inputs = {k: np.asarray(v) for k, v in R.setup_inputs().items()}
    got = kernel(**inputs)
    import jax.numpy as jnp
    ref = np.asarray(
        R.reference(**{k: jnp.asarray(v) for k, v in inputs.items()}))
    err = np.abs(got - ref).max()
    rel = err / np.abs(ref).max()
    print(f"absmax={err:.4e} rel={rel:.4e}")

